# revision 1
# baseline (speedup 1.0000x reference)
"""DINOLoss Trainium2 Bass kernel — 8-core batch-sharded SPMD (v4).

Decomposition (verified vs reference in numpy, rel err ~1e-6):
  loss = [10*(dotTPS_r - dotCSg_r) - 10*(dotTS_r - 2*dotCS_r)
          + (Cn-1)*(Mg - 2*K_tot)] / 1152
with per-core partials all-reduced (TP/SS/Sg vectors + 4 scalars), and
  dotTS_r  = sum_j dot(tp[j], Ssum_{b(j)})   (j = teacher row, replicated Ssum)
  dotTPS_r = sum_j dot(tp[j], s_raw[global row j])
  dotCS_raw  = 0.9*dot(center,SSg) + (1/1280)*dot(TPg,SSg)
  dotCSg_raw = 0.9*dot(center,Sgg) + (1/1280)*dot(TPg,Sgg)
  Cn = 0.9*sum(center) + 0.1

Layouts (per core, d = k4*16384 + m*1024 + q):
  teacher spread: [128, 16384] bf16, partition 32*k4 + j (j<16 live)
  student m-tile: [80, 4096] f32r, free = k4*1024 + q
  psS chunk k4 at psum rows 32*k4+(0..17): 0-15 replicated Ssum_b(j),
      16 = SS, 17 = Sg (cols 18-31 zeroed)
  TP computed via 128 [128x128]@[128x4] slice-matmuls -> LT layout
      stage idx p*512 + s*4 + k4  <->  d = k4*16384 + s*128 + p
  post-pass: SSg/Sgg relayout LS->LT via 8 PE transposes + permuted copies
"""

import sys, os
sys.path.insert(0, "/opt/trn_rl_repo")

import numpy as np
import ml_dtypes

import concourse.bass as bass
import concourse.bacc as bacc
import concourse.tile as tile
import concourse.mybir as mybir
from concourse.bass_utils import run_bass_kernel_spmd

F32 = mybir.dt.float32
F32R = mybir.dt.float32r
BF16 = mybir.dt.bfloat16
AF = mybir.ActivationFunctionType
ALU = mybir.AluOpType
AX = mybir.AxisListType

NCORES = 8
B, G, T = 64, 2, 10
D = 65536
P_S, P_T = 80, 16
SCALE_S, SHIFT_S = 10.0, 45.0
SCALE_T, SHIFT_T = 25.0, 110.0
C = 1024
TDEAD = 5.0                # dead teacher rows filler (exp(25*5-110)=e^15, safe)


def _consts():
    selS1 = np.zeros((P_S, 32), np.float32)
    for j in range(16):
        b = j // 2
        selS1[10 * b:10 * b + 10, j] = 1.0     # replicated Ssum_b at rows 2b,2b+1
    selS1[:, 16] = 1.0                         # SS
    for b in range(8):
        selS1[10 * b:10 * b + 2, 17] = 1.0     # Sg
    selS = np.zeros((P_S, 4, 128), np.float32)
    for k4 in range(4):
        selS[:, k4, 32 * k4:32 * k4 + 32] = selS1
    selS = selS.reshape(P_S, 512)
    selK = np.zeros((128, 4), np.float32)      # TP slice-matmul: col k4 per live row
    for k4 in range(4):
        selK[32 * k4:32 * k4 + 16, k4] = 1.0
    finW = np.zeros((128, 5), np.float32)
    finW[:P_S, 0] = 1.0                        # Lsum
    for b in range(8):
        finW[10 * b:10 * b + 2, 1] = 1.0       # Lg
    for k4 in range(4):
        finW[32 * k4:32 * k4 + 16, 2] = 1.0    # accA live rows
        finW[32 * k4:32 * k4 + 16, 3] = 1.0    # accB live rows
    finW[:, 4] = 1.0                           # post sums (all partitions)
    ident = np.eye(128, dtype=np.float32)
    return selS, selK.astype(ml_dtypes.bfloat16), finW, ident


def build(nc, n_m=16, do_teacher=True, do_coll=True, do_post=True, repeat=1,
          sim_safe=False):
    d_total = D
    TB = d_total // 4

    student = nc.dram_tensor("student_shard", [P_S, d_total], F32, kind="ExternalInput")
    teacher = nc.dram_tensor("teacher_shard", [P_T, d_total], F32, kind="ExternalInput")
    center = nc.dram_tensor("center_full", [1, d_total], F32, kind="ExternalInput")
    out_d = nc.dram_tensor("loss", [1, 1], F32, kind="ExternalOutput")

    selS_np, selK_np, finW_np, ident_np = _consts()
    selS_d = nc.inline_tensor(selS_np, "selS_c")
    selK_d = nc.inline_tensor(np.ascontiguousarray(selK_np), "selK_c")
    finW_d = nc.inline_tensor(finW_np, "finW_c")
    ident_d = nc.inline_tensor(ident_np, "ident_c")

    SC_OFF = 3 * d_total
    STAGE = SC_OFF + 12

    with tile.TileContext(nc) as tc:
        with (
            tc.tile_pool(name="const", bufs=1) as cpool,
            tc.tile_pool(name="acc", bufs=1) as apool,
            tc.tile_pool(name="tch", bufs=1) as tpool,
            tc.tile_pool(name="stu", bufs=2) as spool,
            tc.tile_pool(name="traw", bufs=2) as trawpool,
            tc.tile_pool(name="expo", bufs=1) as epool,
            tc.tile_pool(name="sg", bufs=2) as sgpool,
            tc.tile_pool(name="cp", bufs=2) as cppool,
            tc.tile_pool(name="scr", bufs=1) as scrpool,
            tc.tile_pool(name="post", bufs=1) as ppool,
            tc.tile_pool(name="psum", bufs=1, space=bass.MemorySpace.PSUM) as psp,
            tc.tile_pool(name="dram", bufs=1, space="DRAM") as dpool,
        ):
            selS_sb = cpool.tile([P_S, 512], F32R)
            nc.sync.dma_start(selS_sb[:], selS_d.ap().bitcast(F32R))
            selK_sb = cpool.tile([128, 4], BF16)
            nc.sync.dma_start(selK_sb[:], selK_d.ap())
            finW_sb = cpool.tile([128, 5], F32)
            nc.sync.dma_start(finW_sb[:], finW_d.ap())
            ident_sb = cpool.tile([128, 128], F32)
            nc.sync.dma_start(ident_sb[:], ident_d.ap())
            biasS = cpool.tile([128, 1], F32)
            nc.gpsimd.memset(biasS[:], -SHIFT_S)
            biasT = cpool.tile([128, 1], F32)
            nc.gpsimd.memset(biasT[:], -SHIFT_T)

            sacc = apool.tile([P_S, 16], F32)
            tacc = apool.tile([128, 1], F32)
            accA = apool.tile([128, 16], F32)
            accB = apool.tile([128, 16], F32)
            finacc = apool.tile([128, 3], F32)
            nc.gpsimd.memset(finacc[:], 0.0)
            trow = apool.tile([1, 128], F32)
            s_row = apool.tile([1, 32], F32)
            inv_row = apool.tile([1, 32], F32)
            invb = apool.tile([128, 1], F32)
            er_t = apool.tile([P_S, 1], F32)

            stage_in = dpool.tile([STAGE], F32)
            stage_out = dpool.tile([STAGE], F32)

            tp_sp = tpool.tile([128, TB], BF16)
            sap = student.ap()
            tap = teacher.ap()

            import contextlib
            loop_cm = tc.For_i(0, repeat, 1) if repeat > 1 else contextlib.nullcontext()
            with loop_cm:
                # ---------------- teacher phase (4 slices) ----------------
                TS = TB // 4
                tacc4 = apool.tile([128, 4], F32)
                if not do_teacher:
                    nc.gpsimd.memset(tp_sp[:], 0.001)
                    nc.gpsimd.memset(tacc4[:], 1.0)
                for s in range(4) if do_teacher else []:
                    traw4 = trawpool.tile([128, TS], F32)
                    if sim_safe or s < 2:
                        nc.gpsimd.memset(traw4[:], TDEAD)
                    for k4 in range(4):
                        nc.sync.dma_start(
                            traw4[32 * k4:32 * k4 + 16, :],
                            tap[:, k4 * TB + s * TS:k4 * TB + (s + 1) * TS])
                    nc.scalar.activation(tp_sp[:, s * TS:(s + 1) * TS], traw4[:],
                                         AF.Exp, bias=biasT[:], scale=SCALE_T,
                                         accum_out=tacc4[:, s:s + 1])
                # row sums S_j = sum_k4 sum_s tacc4[32*k4+j, s]
                nc.vector.reduce_sum(tacc[:], tacc4[:], axis=AX.X)
                nc.sync.dma_start(trow[:], tacc[:])
                nc.vector.reduce_sum(s_row[:], trow[:].rearrange(
                    "a (k j) -> a j k", k=4, j=32), axis=AX.X)
                nc.vector.reciprocal(inv_row[:], s_row[:])
                for k4 in range(4):
                    nc.sync.dma_start(invb[32 * k4:32 * k4 + 32, :], inv_row[:])
                nc.vector.tensor_scalar_mul(tp_sp[:], tp_sp[:], invb[:])

                # TP vector in LT layout via 128 tiny slice-matmuls
                psTP = psp.tile([128, 512], F32, tag="psTP")
                for s in range(128):
                    nc.tensor.matmul(psTP[:, 4 * s:4 * s + 4],
                                     tp_sp[:, 128 * s:128 * (s + 1)],
                                     selK_sb[:], start=True, stop=True)
                tp_lt = ppool.tile([128, 512], F32)
                nc.vector.tensor_copy(tp_lt[:], psTP[:])
                nc.sync.dma_start(
                    stage_in[0:d_total].rearrange("(p q) -> p q", p=128), tp_lt[:])

                # ---------------- main pass (16 m-blocks) ----------------
                src = sap.rearrange("r (k mm q) -> r k mm q", k=4, mm=16, q=C)
                srcg = sap.rearrange("(b t) (k mm q) -> b t k mm q",
                                     b=8, t=10, k=4, mm=16, q=C)[:, 0:2]
                for m in range(n_m):
                    sbufF = spool.tile([P_S, 4 * C], F32R)
                    for half in range(2):
                        r0, r1 = 40 * half, 40 * half + 40
                        nc.sync.dma_start(
                            sbufF[r0:r1].rearrange("r (k q) -> r k q", k=4),
                            src[r0:r1, :, m, :].bitcast(F32R))
                    exp_s = epool.tile([P_S, 4 * C], BF16)
                    nc.scalar.activation(exp_s[:], sbufF[:].bitcast(F32), AF.Exp,
                                         bias=biasS[0:P_S], scale=SCALE_S,
                                         accum_out=sacc[:, m:m + 1])
                    psS = psp.tile([128, C], F32, tag="psS", bufs=3)
                    for h in range(2):
                        for k4 in range(4):
                            nc.tensor.matmul(
                                psS[:, 512 * h:512 * h + 512],
                                selS_sb[:, 128 * k4:128 * (k4 + 1)],
                                sbufF[:, k4 * C + 512 * h:k4 * C + 512 * h + 512],
                                start=(k4 == 0), stop=(k4 == 3))
                    ssum = cppool.tile([128, C], F32, tag="ssum")
                    nc.vector.tensor_copy(ssum[:], psS[:])
                    scrA = scrpool.tile([128, C], F32, tag="scrA")
                    nc.vector.scalar_tensor_tensor(
                        scrA[:], tp_sp[:, m * C:(m + 1) * C], 1.0, ssum[:],
                        ALU.mult, ALU.mult, accum_out=accA[:, m:m + 1])
                    # stage SS (row 16) and Sg (row 17)
                    for (row, voff) in ((16, d_total), (17, 2 * d_total)):
                        dst = stage_in[voff:voff + d_total].rearrange(
                            "(k mm q) -> k mm q", k=4, mm=16, q=C)
                        if sim_safe:
                            for jj in range(4):
                                nc.sync.dma_start(
                                    dst[jj, m, :],
                                    ssum[32 * jj + row:32 * jj + row + 1, :])
                        else:
                            nc.sync.dma_start(
                                dst[:, m, :],
                                ssum[:].rearrange("(jj i) q -> jj i q", i=32)[:, row, :])
                    # student-global rows, teacher-spread layout
                    sgdm = sgpool.tile([128, C], F32)
                    if sim_safe or m < 2:
                        nc.gpsimd.memset(sgdm[:], 0.0)
                    for k4 in range(4):
                        nc.gpsimd.dma_start(
                            sgdm[32 * k4:32 * k4 + 16, :],
                            srcg[:, :, k4, m, :])
                    scrB = scrpool.tile([128, C], F32, tag="scrB")
                    nc.vector.scalar_tensor_tensor(
                        scrB[:], tp_sp[:, m * C:(m + 1) * C], 1.0, sgdm[:],
                        ALU.mult, ALU.mult, accum_out=accB[:, m:m + 1])

                # ---------------- finals ----------------
                nc.vector.reduce_sum(er_t[:], sacc[:], axis=AX.X)
                nc.scalar.activation(finacc[0:P_S, 0:1], er_t[:], AF.Ln)
                nc.vector.reduce_sum(finacc[:, 1:2], accA[:], axis=AX.X)
                nc.vector.reduce_sum(finacc[:, 2:3], accB[:], axis=AX.X)
                psfin = psp.tile([4, 3], F32, tag="pstr")
                nc.tensor.matmul(psfin[:], finW_sb[:, 0:4], finacc[:],
                                 start=True, stop=True)
                scl = ppool.tile([4, 3], F32)
                nc.vector.tensor_copy(scl[:], psfin[:])
                nc.sync.dma_start(stage_in[SC_OFF:SC_OFF + 12], scl[:])

            # ---------------- all-reduce ----------------
            if do_coll:
                nc.gpsimd.collective_compute(
                    "AllReduce", ALU.add,
                    replica_groups=[list(range(NCORES))],
                    ins=[stage_in[:].opt()], outs=[stage_out[:].opt()])
            else:
                nc.sync.dma_start(stage_out[:], stage_in[:])

            # ---------------- post pass ----------------
            if do_post:
                PQ = d_total // 128
                TPg = ppool.tile([128, PQ], F32)
                nc.sync.dma_start(TPg[:], stage_out[0:d_total].rearrange("(p q) -> p q", p=128))
                SSg = ppool.tile([128, PQ], F32)
                nc.sync.dma_start(SSg[:], stage_out[d_total:2 * d_total].rearrange("(p q) -> p q", p=128))
                Sgg = ppool.tile([128, PQ], F32)
                nc.sync.dma_start(Sgg[:], stage_out[2 * d_total:3 * d_total].rearrange("(p q) -> p q", p=128))
                cen = ppool.tile([128, PQ], F32)
                nc.sync.dma_start(cen[:], center.ap()[0, :].rearrange("(p q) -> p q", p=128))
                sc_sb = ppool.tile([1, 12], F32)
                nc.sync.dma_start(sc_sb[:], stage_out[SC_OFF:SC_OFF + 12])

                # relayout SSg/Sgg LS->LT: d=pp*512+qq -> LT[qq%128, s*4+k4]
                # per 128-col block c2: transpose, then permuted copy:
                # LT col = (pp%32)*16 + c2*4 + pp//32
                fin2 = ppool.tile([128, 6], F32)
                nc.gpsimd.memset(fin2[:], 0.0)
                scrP = ppool.tile([128, PQ], F32)
                for (vec, col_d1, col_d2) in ((SSg, 0, 1), (Sgg, 2, 3)):
                    v_lt = ppool.tile([128, 512], F32, tag="vlt")
                    for c2 in range(4):
                        pst = psp.tile([128, 128], F32, tag="pstr")
                        nc.tensor.transpose(
                            pst[:], vec[:, 128 * c2:128 * (c2 + 1)], ident_sb[:])
                        nc.vector.tensor_copy(
                            v_lt[:].rearrange("p (a x b) -> p a x b", a=32, x=4, b=4)
                            [:, :, c2, :],
                            pst[:].rearrange("p (b a) -> p a b", b=4, a=32))
                    # dot(center, vec) in LS; dot(TPg, vec_LT) in LT
                    nc.vector.scalar_tensor_tensor(
                        scrP[:], cen[:], 1.0, vec[:], ALU.mult, ALU.mult,
                        accum_out=fin2[:, col_d1:col_d1 + 1])
                    nc.vector.scalar_tensor_tensor(
                        scrP[:, 0:512], TPg[:, 0:512], 1.0, v_lt[:],
                        ALU.mult, ALU.mult,
                        accum_out=fin2[:, col_d2:col_d2 + 1])
                nc.vector.reduce_sum(fin2[:, 4:5], cen[:], axis=AX.X)
                psf2 = psp.tile([1, 6], F32, tag="pstr")
                nc.tensor.matmul(psf2[:], finW_sb[:, 4:5], fin2[:],
                                 start=True, stop=True)
                f2 = ppool.tile([1, 6], F32)
                nc.vector.tensor_copy(f2[:], psf2[:])

                # scalar arithmetic on partition 0
                fs = ppool.tile([1, 16], F32)
                n_rows_s = NCORES * P_S
                n_rows_g = NCORES * P_T
                # dotCS_raw = 0.9*dCS1 + (1/1280)*dCS2 ; same for Sg
                nc.vector.tensor_scalar_mul(fs[:, 12:13], f2[:, 0:1], 0.9)
                nc.vector.scalar_tensor_tensor(
                    fs[:, 13:14], f2[:, 1:2], 1.0 / 1280.0, fs[:, 12:13],
                    ALU.mult, ALU.add)                      # dotCS_raw
                nc.vector.tensor_scalar_mul(fs[:, 14:15], f2[:, 2:3], 0.9)
                nc.vector.scalar_tensor_tensor(
                    fs[:, 15:16], f2[:, 3:4], 1.0 / 1280.0, fs[:, 14:15],
                    ALU.mult, ALU.add)                      # dotCSg_raw
                # Cn = 0.9*Cc + 0.1
                nc.vector.tensor_scalar(fs[:, 11:12], f2[:, 4:5], 0.9, 0.1,
                                        ALU.mult, ALU.add)
                # t1 = dotTPS_r - dotCSg_r
                nc.vector.tensor_tensor(fs[:, 0:1], sc_sb[:, 11:12], fs[:, 15:16],
                                        ALU.subtract)
                # t2 = dotTS_r - 2*dotCS_r
                nc.vector.tensor_scalar_mul(fs[:, 1:2], fs[:, 13:14], 2.0)
                nc.vector.tensor_tensor(fs[:, 2:3], sc_sb[:, 7:8], fs[:, 1:2],
                                        ALU.subtract)
                # t3 = t1 - t2
                nc.vector.tensor_tensor(fs[:, 3:4], fs[:, 0:1], fs[:, 2:3],
                                        ALU.subtract)
                # cn1 = Cn - 1
                nc.vector.tensor_scalar_add(fs[:, 4:5], fs[:, 11:12], -1.0)
                # m2k = (Lg' + n_g*SHIFT) - 2*(Lsum' + n_s*SHIFT)
                nc.vector.tensor_scalar_mul(fs[:, 5:6], sc_sb[:, 0:1], 2.0)
                nc.vector.tensor_tensor(fs[:, 6:7], sc_sb[:, 3:4], fs[:, 5:6],
                                        ALU.subtract)
                nc.vector.tensor_scalar_add(
                    fs[:, 7:8], fs[:, 6:7],
                    float(n_rows_g * SHIFT_S - 2 * n_rows_s * SHIFT_S))
                # t4 = cn1 * m2k ; t5 = 10*t3 ; loss = (t5+t4)/1152
                nc.vector.tensor_tensor(fs[:, 8:9], fs[:, 4:5], fs[:, 7:8], ALU.mult)
                nc.vector.tensor_scalar_mul(fs[:, 9:10], fs[:, 3:4], SCALE_S)
                nc.vector.tensor_tensor(fs[:, 10:11], fs[:, 9:10], fs[:, 8:9], ALU.add)
                nc.vector.tensor_scalar_mul(fs[:, 11:12], fs[:, 10:11],
                                            1.0 / float(B * G * (T - 1)))
                nc.sync.dma_start(out_d.ap(), fs[:, 11:12])
            else:
                dummy = ppool.tile([1, 1], F32)
                nc.sync.dma_start(dummy[:], stage_out[0:1])
                nc.sync.dma_start(out_d.ap(), dummy[:])
    nc.compile()
    return nc


_CACHE = {}


def _get_nc():
    if "nc" not in _CACHE:
        nc = bacc.Bacc("TRN2", target_bir_lowering=False, debug=False,
                       enable_asserts=False, num_devices=NCORES)
        _CACHE["nc"] = build(nc)
    return _CACHE["nc"]


def kernel(student_output, teacher_output, center, batch_size=64, epoch=0):
    nc = _get_nc()
    student = np.ascontiguousarray(np.asarray(student_output, dtype=np.float32))
    teacher = np.ascontiguousarray(np.asarray(teacher_output, dtype=np.float32))
    cen = np.ascontiguousarray(np.asarray(center, dtype=np.float32))
    in_maps = []
    for c in range(NCORES):
        in_maps.append({
            "student_shard": np.ascontiguousarray(student[P_S * c:P_S * (c + 1)]),
            "teacher_shard": np.ascontiguousarray(teacher[P_T * c:P_T * (c + 1)]),
            "center_full": cen,
        })
    res = run_bass_kernel_spmd(nc, in_maps, core_ids=list(range(NCORES)))
    _CACHE["last_result"] = res
    return np.asarray(res.results[0]["loss"], np.float32).reshape(1)


if __name__ == "__main__":
    import reference
    inputs = reference.setup_inputs()
    expected = np.array(reference.reference(**inputs))
    actual = kernel(**{k: np.asarray(v) for k, v in inputs.items()})
    rel = abs(actual[0] - expected[0]) / abs(expected[0])
    print("expected", expected, "actual", actual, "rel", rel)



# revision 25
# speedup vs baseline: 3.2950x; 3.2950x over previous
"""DINOLoss Trainium2 Bass kernel — 8-core batch-sharded SPMD (v5).

Decomposition (verified vs reference in numpy, rel err ~1e-6):
  loss = [10*(dotTPS_r - dotCSg_r) - 10*(dotTS_r - 2*dotCS_r)
          + (Cn-1)*(Mg - 2*K_tot)] / 1152
with per-core partials all-reduced (TP/SS/Sg vectors + 4 scalars), and
  dotTS_r  = sum_j dot(tp[j], Ssum_{b(j)})   (j = teacher row, replicated Ssum)
  dotTPS_r = sum_j dot(tp[j], s_raw[global row j])
  dotCS_raw  = 0.9*dot(center,SSg) + (1/1280)*dot(TPg,SSg)
  dotCSg_raw = 0.9*dot(center,Sgg) + (1/1280)*dot(TPg,Sgg)
  Cn = 0.9*sum(center) + 0.1

v5 pipeline changes vs v4:
  - postponed softmax normalization: teacher exp stays raw (e_t); accA/accB
    rows and selK are scaled by invb = 1/S_j at the END (removes the
    teacher-phase barrier; teacher slices interleave with the m-loop).
  - teacher dead rows filled with 4.4 so exp(25*4.4-110)=1.0; scrA output
    rows 16/17 of each 32-block then equal SS/Sg m-chunks exactly, so SS/Sg
    staging DMAs read scrA's own (otherwise scratch) output -> the separate
    psS->SBUF ssum copy is gone.
  - scrA reads psS straight from PSUM; scrB runs on Pool (gpsimd) reading
    sgdm SBUF, freeing DVE.
  - merged DMAs: student 1/m, sgdm 1/m (HWDGE, not SWDGE), teacher 1/slice,
    SS+Sg staging 1/m.
  - teacher raw tiles memset once outside the loop (dead rows stay 4.4).

Layouts (per core, d = k4*16384 + m*1024 + q):
  teacher spread: [128, 16384] bf16, partition 32*k4 + j (j<16 live)
  student m-tile: [80, 4096] f32r, free = k4*1024 + q
  psS chunk k4 at psum rows 32*k4+(0..17): 0-15 replicated Ssum_b(j),
      16 = SS, 17 = Sg
  TP computed via 128 [128x128]@[128x4] slice-matmuls -> LT layout
      stage idx p*512 + s*4 + k4  <->  d = k4*16384 + s*128 + p
  post-pass: SSg/Sgg relayout LS->LT via 8 PE transposes + permuted copies
"""

import sys, os
sys.path.insert(0, "/opt/trn_rl_repo")

import numpy as np
import ml_dtypes

import concourse.bass as bass
import concourse.bacc as bacc
import concourse.tile as tile
import concourse.mybir as mybir
from concourse.bass_utils import run_bass_kernel_spmd

F32 = mybir.dt.float32
F32R = mybir.dt.float32r
BF16 = mybir.dt.bfloat16
AF = mybir.ActivationFunctionType
ALU = mybir.AluOpType
AX = mybir.AxisListType

NCORES = 8
B, G, T = 64, 2, 10
D = 65536
P_S, P_T = 80, 16
SCALE_S, SHIFT_S = 10.0, 45.0
SCALE_T, SHIFT_T = 25.0, 110.0
C = 1024
TDEAD = 4.4                # dead teacher rows: exp(25*4.4-110)=1.0 exactly


def _consts():
    selS1 = np.zeros((P_S, 32), np.float32)
    for j in range(16):
        b = j // 2
        selS1[10 * b:10 * b + 10, j] = 1.0     # replicated Ssum_b at rows 2b,2b+1
    selS1[:, 16] = 1.0                         # SS
    for b in range(8):
        selS1[10 * b:10 * b + 2, 17] = 1.0     # Sg
    selS = np.zeros((P_S, 4, 128), np.float32)
    for k4 in range(4):
        selS[:, k4, 32 * k4:32 * k4 + 32] = selS1
    selS = selS.reshape(P_S, 512)
    selK = np.zeros((128, 4), np.float32)      # TP slice-matmul: col k4 per live row
    for k4 in range(4):
        selK[32 * k4:32 * k4 + 16, k4] = 1.0
    finW = np.zeros((128, 5), np.float32)
    finW[:P_S, 0] = 1.0                        # Lsum
    for b in range(8):
        finW[10 * b:10 * b + 2, 1] = 1.0       # Lg
    for k4 in range(4):
        finW[32 * k4:32 * k4 + 16, 2] = 1.0    # accA live rows
        finW[32 * k4:32 * k4 + 16, 3] = 1.0    # accB live rows
    finW[:, 4] = 1.0                           # post sums (all partitions)
    ident = np.eye(128, dtype=np.float32)
    return selS, selK.astype(ml_dtypes.bfloat16), finW, ident


def build(nc, n_m=16, do_teacher=True, do_coll=True, do_post=True, repeat=1,
          sim_safe=False, swdge=7):
    d_total = D
    TB = d_total // 4
    TS = TB // 4                               # 4096 = 4 m-blocks

    student = nc.dram_tensor("student_shard", [P_S, d_total], F32, kind="ExternalInput")
    teacher = nc.dram_tensor("teacher_shard", [P_T, d_total], F32, kind="ExternalInput")
    center = nc.dram_tensor("center_full", [1, d_total], F32, kind="ExternalInput")
    out_d = nc.dram_tensor("loss", [1, 1], F32, kind="ExternalOutput")

    selS_np, selK_np, finW_np, ident_np = _consts()
    selS_d = nc.inline_tensor(selS_np, "selS_c")
    selK_d = nc.inline_tensor(np.ascontiguousarray(selK_np), "selK_c")
    finW_d = nc.inline_tensor(finW_np, "finW_c")
    ident_d = nc.inline_tensor(ident_np, "ident_c")

    SC_OFF = 3 * d_total
    STAGE = SC_OFF + 12

    with tile.TileContext(nc) as tc:
        with (
            tc.tile_pool(name="const", bufs=1) as cpool,
            tc.tile_pool(name="acc", bufs=1) as apool,
            tc.tile_pool(name="tch", bufs=1) as tpool,
            tc.tile_pool(name="stu", bufs=4) as spool,
            tc.tile_pool(name="traw", bufs=1) as trawpool,
            tc.tile_pool(name="expo", bufs=1) as epool,
            tc.tile_pool(name="sg", bufs=3) as sgpool,
            tc.tile_pool(name="scr", bufs=1) as scrpool,
            tc.tile_pool(name="post", bufs=1) as ppool,
            tc.tile_pool(name="psum", bufs=1, space=bass.MemorySpace.PSUM) as psp,
            tc.tile_pool(name="dram", bufs=1, space="DRAM") as dpool,
        ):
            selS_sb = cpool.tile([P_S, 512], F32R)
            nc.sync.dma_start(selS_sb[:], selS_d.ap().bitcast(F32R))
            selK_sb = cpool.tile([128, 4], BF16)
            nc.sync.dma_start(selK_sb[:], selK_d.ap())
            finW_sb = cpool.tile([128, 5], F32)
            nc.sync.dma_start(finW_sb[:], finW_d.ap())
            ident_sb = cpool.tile([128, 128], F32)
            nc.sync.dma_start(ident_sb[:], ident_d.ap())
            biasS = cpool.tile([128, 1], F32)
            nc.gpsimd.memset(biasS[:], -SHIFT_S)
            biasT = cpool.tile([128, 1], F32)
            nc.gpsimd.memset(biasT[:], -SHIFT_T)

            sacc = apool.tile([P_S, 16], F32)
            tacc = apool.tile([128, 1], F32)
            tacc4 = apool.tile([128, 4], F32)
            accA = apool.tile([128, 16], F32)
            accB = apool.tile([128, 16], F32)
            finacc = apool.tile([128, 3], F32)
            nc.gpsimd.memset(finacc[:], 0.0)
            trow = apool.tile([1, 128], F32)
            s_row = apool.tile([1, 32], F32)
            inv_row = apool.tile([1, 32], F32)
            invb = apool.tile([128, 1], F32)
            er_t = apool.tile([P_S, 1], F32)
            accAr = apool.tile([128, 1], F32)
            accBr = apool.tile([128, 1], F32)
            selKs = apool.tile([128, 4], BF16)

            stage_in = dpool.tile([STAGE], F32)
            stage_out = dpool.tile([STAGE], F32)

            tp_sp = tpool.tile([128, TB], BF16)
            sap = student.ap()
            tap = teacher.ap()

            # teacher raw tiles: 4 resident slices, dead rows set once to
            # TDEAD (exp -> 1.0) and never touched again
            traws = [trawpool.tile([128, TS], F32, tag=f"traw{i}",
                                   name=f"traw{i}") for i in range(3)]
            traws.append(traws[0])             # slice 3 reuses slice 0's buffer
            nc.gpsimd.memset(traws[0][:], TDEAD)
            # sgdm dead rows stay 0 forever
            sgds = [sgpool.tile([128, C], F32, tag="sgdm", name=f"sgdm{i}")
                    for i in range(3)]
            for t in sgds:
                nc.gpsimd.memset(t[:], 0.0)
            nc.vector.memset(traws[1][:], TDEAD)
            nc.vector.memset(traws[2][:], TDEAD)

            tsrc = tap.rearrange("j (k s q) -> k j s q", k=4, s=4, q=TS)
            src = sap.rearrange("r (k mm q) -> r k mm q", k=4, mm=16, q=C)
            srcg = sap.rearrange("(b t) (k mm q) -> b t k mm q",
                                 b=8, t=10, k=4, mm=16, q=C)[:, 0:2]
            stg = stage_in[d_total:3 * d_total].rearrange(
                "(v k mm q) -> k v mm q", v=2, k=4, mm=16, q=C)

            import contextlib
            loop_cm = tc.For_i(0, repeat, 1) if repeat > 1 else contextlib.nullcontext()
            with loop_cm:
                if not do_teacher:
                    nc.gpsimd.memset(tp_sp[:], 0.001)
                    nc.gpsimd.memset(tacc4[:], 1.0)
                else:
                    for k4 in range(4):
                        (nc.gpsimd if swdge & 1 else nc.sync).dma_start(
                            traws[0][32 * k4:32 * k4 + 16, :],
                            tsrc[k4, :, 0, :])

                for m in range(n_m):
                    # front-loaded teacher: exp slice m at m=0..3, prefetch next
                    if do_teacher and m < 4:
                        if m + 1 < 4:
                            for k4 in range(4):
                                (nc.gpsimd if swdge & 1 else nc.sync).dma_start(
                                    traws[m + 1][32 * k4:32 * k4 + 16, :],
                                    tsrc[k4, :, m + 1, :])
                        nc.scalar.activation(
                            tp_sp[:, m * TS:(m + 1) * TS], traws[m][:],
                            AF.Exp, bias=biasT[:], scale=SCALE_T,
                            accum_out=tacc4[:, m:m + 1])
                    if do_teacher and m == 6:
                        # teacher row sums -> invb -> selKs (DVE/Pool idle here)
                        nc.vector.reduce_sum(tacc[:], tacc4[:], axis=AX.X)
                        nc.sync.dma_start(trow[:], tacc[:])
                        nc.vector.reduce_sum(s_row[:], trow[:].rearrange(
                            "a (k j) -> a j k", k=4, j=32), axis=AX.X)
                        nc.vector.reciprocal(inv_row[:], s_row[:])
                        for k4 in range(4):
                            nc.sync.dma_start(
                                invb[32 * k4:32 * k4 + 32, :], inv_row[:])
                        nc.vector.tensor_scalar_mul(selKs[:], selK_sb[:], invb[:])
                    if m == 7:
                        psTP = psp.tile([128, 512], F32, tag="psTP")
                        tp_lt = ppool.tile([128, 512], F32)
                    if 7 <= m < 15:
                        # TP slice-matmuls spread over the loop (PE SEQ slack)
                        for s2 in range(16 * (m - 7), 16 * (m - 6)):
                            nc.tensor.matmul(psTP[:, 4 * s2:4 * s2 + 4],
                                             tp_sp[:, 128 * s2:128 * (s2 + 1)],
                                             selKs[:], start=True, stop=True)
                    if m == 15:
                        nc.vector.tensor_copy(tp_lt[:], psTP[:])
                        (nc.gpsimd if swdge & 4 else nc.sync).dma_start(
                            stage_in[0:d_total].rearrange("(p q) -> p q", p=128),
                            tp_lt[:])

                    sbufF = spool.tile([P_S, 4 * C], F32R)
                    nc.sync.dma_start(
                        sbufF[:].rearrange("r (k q) -> r k q", k=4),
                        src[:, :, m, :].bitcast(F32R))
                    exp_s = epool.tile([P_S, 4 * C], BF16)
                    nc.scalar.activation(exp_s[:], sbufF[:].bitcast(F32), AF.Exp,
                                         bias=biasS[0:P_S], scale=SCALE_S,
                                         accum_out=sacc[:, m:m + 1])
                    psS = psp.tile([128, C], F32, tag="psS", bufs=3)
                    for h in range(2):
                        for k4 in range(4):
                            nc.tensor.matmul(
                                psS[:, 512 * h:512 * h + 512],
                                selS_sb[:, 128 * k4:128 * (k4 + 1)],
                                sbufF[:, k4 * C + 512 * h:k4 * C + 512 * h + 512],
                                start=(k4 == 0), stop=(k4 == 3))
                    # student-global rows re-read in teacher-spread layout
                    sgdm = sgds[m % 3]
                    for k4 in range(4):
                        nc.sync.dma_start(
                            sgdm[32 * k4:32 * k4 + 16, :],
                            srcg[:, :, k4, m, :])
                    scrB = scrpool.tile([128, C], F32, tag="scrB", bufs=1)
                    nc.vector.scalar_tensor_tensor(
                        scrB[:], tp_sp[:, m * C:(m + 1) * C], 1.0, sgdm[:],
                        ALU.mult, ALU.mult, accum_out=accB[:, m:m + 1])
                    # scrA = tp_raw * psS (PSUM direct); rows 16/17 of each
                    # 32-block = 1.0 * SS/Sg -> staged below from scrA itself
                    scrA = scrpool.tile([128, C], F32, tag="scrA", bufs=3)
                    nc.vector.scalar_tensor_tensor(
                        scrA[:], tp_sp[:, m * C:(m + 1) * C], 1.0, psS[:],
                        ALU.mult, ALU.mult, accum_out=accA[:, m:m + 1])
                    for v, row in ((0, 16), (1, 17)):
                        if sim_safe:
                            for jj in range(4):
                                nc.sync.dma_start(
                                    stg[jj, v, m, :],
                                    scrA[32 * jj + row:32 * jj + row + 1, :])
                        else:
                            (nc.gpsimd if swdge & 2 else nc.sync).dma_start(
                                stg[:, v, m, :],
                                scrA[:].rearrange("(jj i) q -> jj i q",
                                                  i=32)[:, row, :])

                # ---------------- finals ----------------
                nc.vector.reduce_sum(er_t[:], sacc[:], axis=AX.X)
                nc.scalar.activation(finacc[0:P_S, 0:1], er_t[:], AF.Ln)
                nc.vector.reduce_sum(accAr[:], accA[:], axis=AX.X)
                nc.vector.tensor_tensor(finacc[:, 1:2], accAr[:], invb[:], ALU.mult)
                nc.vector.reduce_sum(accBr[:], accB[:], axis=AX.X)
                nc.vector.tensor_tensor(finacc[:, 2:3], accBr[:], invb[:], ALU.mult)
                psfin = psp.tile([4, 3], F32, tag="pstr")
                nc.tensor.matmul(psfin[:], finW_sb[:, 0:4], finacc[:],
                                 start=True, stop=True)
                scl = ppool.tile([4, 3], F32)
                nc.vector.tensor_copy(scl[:], psfin[:])
                nc.sync.dma_start(stage_in[SC_OFF:SC_OFF + 12], scl[:])

            # ---------------- all-reduce ----------------
            if do_coll:
                nc.gpsimd.collective_compute(
                    "AllReduce", ALU.add,
                    replica_groups=[list(range(NCORES))],
                    ins=[stage_in[:].opt()], outs=[stage_out[:].opt()])
            else:
                nc.sync.dma_start(stage_out[:], stage_in[:])

            # ---------------- post pass ----------------
            if do_post:
                PQ = d_total // 128
                TPg = ppool.tile([128, PQ], F32)
                nc.sync.dma_start(TPg[:], stage_out[0:d_total].rearrange("(p q) -> p q", p=128))
                SSg = ppool.tile([128, PQ], F32)
                nc.sync.dma_start(SSg[:], stage_out[d_total:2 * d_total].rearrange("(p q) -> p q", p=128))
                Sgg = ppool.tile([128, PQ], F32)
                nc.sync.dma_start(Sgg[:], stage_out[2 * d_total:3 * d_total].rearrange("(p q) -> p q", p=128))
                cen = ppool.tile([128, PQ], F32)
                nc.sync.dma_start(cen[:], center.ap()[0, :].rearrange("(p q) -> p q", p=128))
                sc_sb = ppool.tile([1, 12], F32)
                nc.sync.dma_start(sc_sb[:], stage_out[SC_OFF:SC_OFF + 12])

                # relayout SSg/Sgg LS->LT: d=pp*512+qq -> LT[qq%128, s*4+k4]
                fin2 = ppool.tile([128, 6], F32)
                nc.gpsimd.memset(fin2[:], 0.0)
                scrP = ppool.tile([128, PQ], F32)
                for (vec, col_d1, col_d2) in ((SSg, 0, 1), (Sgg, 2, 3)):
                    v_lt = ppool.tile([128, 512], F32, tag="vlt")
                    for c2 in range(4):
                        pst = psp.tile([128, 128], F32, tag="pstr")
                        nc.tensor.transpose(
                            pst[:], vec[:, 128 * c2:128 * (c2 + 1)], ident_sb[:])
                        nc.vector.tensor_copy(
                            v_lt[:].rearrange("p (a x b) -> p a x b", a=32, x=4, b=4)
                            [:, :, c2, :],
                            pst[:].rearrange("p (b a) -> p a b", b=4, a=32))
                    nc.vector.scalar_tensor_tensor(
                        scrP[:], cen[:], 1.0, vec[:], ALU.mult, ALU.mult,
                        accum_out=fin2[:, col_d1:col_d1 + 1])
                    nc.vector.scalar_tensor_tensor(
                        scrP[:, 0:512], TPg[:, 0:512], 1.0, v_lt[:],
                        ALU.mult, ALU.mult,
                        accum_out=fin2[:, col_d2:col_d2 + 1])
                nc.vector.reduce_sum(fin2[:, 4:5], cen[:], axis=AX.X)
                psf2 = psp.tile([1, 6], F32, tag="pstr")
                nc.tensor.matmul(psf2[:], finW_sb[:, 4:5], fin2[:],
                                 start=True, stop=True)
                f2 = ppool.tile([1, 6], F32)
                nc.vector.tensor_copy(f2[:], psf2[:])

                # scalar arithmetic on partition 0
                fs = ppool.tile([1, 16], F32)
                n_rows_s = NCORES * P_S
                n_rows_g = NCORES * P_T
                # dotCS_raw = 0.9*dCS1 + (1/1280)*dCS2 ; same for Sg
                nc.vector.tensor_scalar_mul(fs[:, 12:13], f2[:, 0:1], 0.9)
                nc.vector.scalar_tensor_tensor(
                    fs[:, 13:14], f2[:, 1:2], 1.0 / 1280.0, fs[:, 12:13],
                    ALU.mult, ALU.add)                      # dotCS_raw
                nc.vector.tensor_scalar_mul(fs[:, 14:15], f2[:, 2:3], 0.9)
                nc.vector.scalar_tensor_tensor(
                    fs[:, 15:16], f2[:, 3:4], 1.0 / 1280.0, fs[:, 14:15],
                    ALU.mult, ALU.add)                      # dotCSg_raw
                # Cn = 0.9*Cc + 0.1
                nc.vector.tensor_scalar(fs[:, 11:12], f2[:, 4:5], 0.9, 0.1,
                                        ALU.mult, ALU.add)
                # t1 = dotTPS_r - dotCSg_r
                nc.vector.tensor_tensor(fs[:, 0:1], sc_sb[:, 11:12], fs[:, 15:16],
                                        ALU.subtract)
                # t2 = dotTS_r - 2*dotCS_r
                nc.vector.tensor_scalar_mul(fs[:, 1:2], fs[:, 13:14], 2.0)
                nc.vector.tensor_tensor(fs[:, 2:3], sc_sb[:, 7:8], fs[:, 1:2],
                                        ALU.subtract)
                # t3 = t1 - t2
                nc.vector.tensor_tensor(fs[:, 3:4], fs[:, 0:1], fs[:, 2:3],
                                        ALU.subtract)
                # cn1 = Cn - 1
                nc.vector.tensor_scalar_add(fs[:, 4:5], fs[:, 11:12], -1.0)
                # m2k = (Lg' + n_g*SHIFT) - 2*(Lsum' + n_s*SHIFT)
                nc.vector.tensor_scalar_mul(fs[:, 5:6], sc_sb[:, 0:1], 2.0)
                nc.vector.tensor_tensor(fs[:, 6:7], sc_sb[:, 3:4], fs[:, 5:6],
                                        ALU.subtract)
                nc.vector.tensor_scalar_add(
                    fs[:, 7:8], fs[:, 6:7],
                    float(n_rows_g * SHIFT_S - 2 * n_rows_s * SHIFT_S))
                # t4 = cn1 * m2k ; t5 = 10*t3 ; loss = (t5+t4)/1152
                nc.vector.tensor_tensor(fs[:, 8:9], fs[:, 4:5], fs[:, 7:8], ALU.mult)
                nc.vector.tensor_scalar_mul(fs[:, 9:10], fs[:, 3:4], SCALE_S)
                nc.vector.tensor_tensor(fs[:, 10:11], fs[:, 9:10], fs[:, 8:9], ALU.add)
                nc.vector.tensor_scalar_mul(fs[:, 11:12], fs[:, 10:11],
                                            1.0 / float(B * G * (T - 1)))
                nc.sync.dma_start(out_d.ap(), fs[:, 11:12])
            else:
                dummy = ppool.tile([1, 1], F32)
                nc.sync.dma_start(dummy[:], stage_out[0:1])
                nc.sync.dma_start(out_d.ap(), dummy[:])
    nc.compile()
    return nc


_CACHE = {}


def _get_nc():
    if "nc" not in _CACHE:
        nc = bacc.Bacc("TRN2", target_bir_lowering=False, debug=False,
                       enable_asserts=False, num_devices=NCORES)
        _CACHE["nc"] = build(nc)
    return _CACHE["nc"]


def kernel(student_output, teacher_output, center, batch_size=64, epoch=0):
    nc = _get_nc()
    student = np.ascontiguousarray(np.asarray(student_output, dtype=np.float32))
    teacher = np.ascontiguousarray(np.asarray(teacher_output, dtype=np.float32))
    cen = np.ascontiguousarray(np.asarray(center, dtype=np.float32))
    in_maps = []
    for c in range(NCORES):
        in_maps.append({
            "student_shard": np.ascontiguousarray(student[P_S * c:P_S * (c + 1)]),
            "teacher_shard": np.ascontiguousarray(teacher[P_T * c:P_T * (c + 1)]),
            "center_full": cen,
        })
    res = run_bass_kernel_spmd(nc, in_maps, core_ids=list(range(NCORES)))
    _CACHE["last_result"] = res
    return np.asarray(res.results[0]["loss"], np.float32).reshape(1)


if __name__ == "__main__":
    import reference
    inputs = reference.setup_inputs()
    expected = np.array(reference.reference(**inputs))
    actual = kernel(**{k: np.asarray(v) for k, v in inputs.items()})
    rel = abs(actual[0] - expected[0]) / abs(expected[0])
    print("expected", expected, "actual", actual, "rel", rel)
